# revision 11
# baseline (speedup 1.0000x reference)
"""Bit2Num dequantization kernel for Trainium2 (Bass/Tile), SPMD over 8 cores.

Reference computation (B=4):
    bits = x.reshape(batch, 2048, 4)                # x in {0,1} stored fp32
    num  = sum_b bits[..., b] * 2**(3-b)            # weights [8,4,2,1]
    out  = (num + 0.5) / 16
        = 0.5*x0 + 0.25*x1 + 0.125*x2 + 0.0625*x3 + 0.03125

Sharding: batch (16384) split evenly across 8 NeuronCores; pure data
parallel, no collectives.

Per-core kernel: 16 stripes of [128 rows x 8192 cols]. Each stripe is one
contiguous 4MB DMA load; the 4 bit-streams are strided SBUF views
(stride 4). Compute is a Horner chain:
    s3 = 0.0625 * x3                      (ScalarE, free affine)
    u  = (x2 * 0.125 + 0.03125) + s3      (VectorE AFFINE_THEN_ADD)
    v  = (x1 * 0.25) + u                  (VectorE AFFINE_THEN_ADD)
    o  = (x0 * 0.5)  + v                  (VectorE AFFINE_THEN_ADD)
All values are dyadic rationals representable exactly in fp32, so the
result is bit-exact vs the reference.
"""

import numpy as np

BATCH = 16384
N_SYM = 2048
NBITS = 4
COLS = N_SYM * NBITS  # 8192
N_CORES = 8
ROWS_PER_CORE = BATCH // N_CORES  # 2048
P = 128  # SBUF partitions

_NC_CACHE = {}


DEFAULT_CHUNK = 4096


def _build_program(col_chunk=DEFAULT_CHUNK, repeats=1):
    """Build the per-core Bass program (identical on every core).

    repeats>1 re-runs the whole computation N times inside one NEFF —
    used only for benchmarking (launch overhead cancels in T(N)-T(1))."""
    import concourse.mybir as mybir
    from concourse import bacc
    from concourse.tile import TileContext

    # Bacc (not raw Bass): its compile() pass splits multi-sem waits into
    # event-semaphore chains (TRN2 allows max 1 wait/instruction) and runs
    # codegen for extended-ISA instructions (the custom DVE op below).
    nc = bacc.Bacc("TRN2")
    f32 = mybir.dt.float32
    x = nc.dram_tensor("x", [ROWS_PER_CORE, COLS], f32, kind="ExternalInput")
    out = nc.dram_tensor("out", [ROWS_PER_CORE, N_SYM], f32, kind="ExternalOutput")

    n_stripes = ROWS_PER_CORE // P  # 16
    chunks_per_stripe = COLS // col_chunk
    sym_chunk = col_chunk // NBITS

    with TileContext(nc) as tc:
        with (
            tc.tile_pool(name="inp", bufs=3) as in_pool,
            tc.tile_pool(name="mid", bufs=3) as mid_pool,
            tc.tile_pool(name="outp", bufs=3) as out_pool,
        ):
            for i in [
                s for _ in range(repeats) for s in range(n_stripes)
            ]:
                for c in range(chunks_per_stripe):
                    xt = in_pool.tile([P, col_chunk], f32, tag="xt")
                    nc.sync.dma_start(
                        out=xt,
                        in_=x[i * P : (i + 1) * P, c * col_chunk : (c + 1) * col_chunk],
                    )
                    xb = xt.rearrange("p (s b) -> p s b", b=NBITS)
                    x0, x1, x2, x3 = (xb[:, :, b] for b in range(NBITS))

                    # Horner over raw streams only (keeps each custom DVE op
                    # at <=1 cross-engine wait): w = x0 + x1/2 + x2/4 + x3/8
                    u = mid_pool.tile([P, sym_chunk], f32, tag="u")
                    nc.vector.affine_then_add(
                        out=u, in0=x3, in1=x2, scale=0.5, bias=0.0
                    )
                    v = mid_pool.tile([P, sym_chunk], f32, tag="v")
                    nc.vector.affine_then_add(
                        out=v, in0=u, in1=x1, scale=0.5, bias=0.0
                    )
                    w = mid_pool.tile([P, sym_chunk], f32, tag="w")
                    nc.vector.affine_then_add(
                        out=w, in0=v, in1=x0, scale=0.5, bias=0.0
                    )
                    # o = w/2 + 1/32 = (num + 0.5)/16, on ScalarE (normal
                    # ACTIVATE, fine with multiple sem waits)
                    o = out_pool.tile([P, sym_chunk], f32, tag="o")
                    nc.scalar.activation(
                        o,
                        w,
                        mybir.ActivationFunctionType.Copy,
                        bias=0.03125,
                        scale=0.5,
                    )
                    nc.sync.dma_start(
                        out=out[
                            i * P : (i + 1) * P, c * sym_chunk : (c + 1) * sym_chunk
                        ],
                        in_=o,
                    )

    nc.finalize()
    return nc


def _get_nc(col_chunk=DEFAULT_CHUNK):
    if col_chunk not in _NC_CACHE:
        _NC_CACHE[col_chunk] = _build_program(col_chunk)
    return _NC_CACHE[col_chunk]


def run(x, trace=False, col_chunk=DEFAULT_CHUNK):
    """Run the SPMD kernel; returns (full_output, BassKernelResults)."""
    from concourse.bass_utils import run_bass_kernel_spmd

    x = np.asarray(x, dtype=np.float32)
    assert x.shape == (BATCH, COLS), x.shape
    nc = _get_nc(col_chunk)
    shards = np.split(x, N_CORES, axis=0)
    in_maps = [{"x": np.ascontiguousarray(s)} for s in shards]
    res = run_bass_kernel_spmd(
        nc, in_maps, core_ids=list(range(N_CORES)), trace=trace
    )
    out = np.concatenate([r["out"] for r in res.results], axis=0)
    return out, res


def kernel(x, B=4, **_ignored):
    assert int(B) == NBITS
    out, _ = run(x, trace=False)
    return out
